# revision 1
# baseline (speedup 1.0000x reference)
"""Multi-head attention Bass/Tile kernel for Trainium2, 8 cores data-parallel.

Shapes (hardcoded): x [8, 1024, 768], Wqkv [768, 2304], bqkv [2304],
Wproj [768, 768], bproj [768].  B=8 batches -> one batch per NeuronCore.

Per-core dataflow (all matmul operands fp16, PSUM accumulation fp32):
  qT/kT [c, n]  : stationary = W-tiles, moving = xT       (c = head-padded 8*128)
  v     [n, c'] : stationary = xT-tiles, moving = Wv_aug  (c' = 8*(96+1), ones col)
  S^T   [j, i]  : stationary = kT head tile, moving = qT head tile
  expS^T        : ACT exp with fused *E^-0.5 scale, psum->sbuf fp16
  o_aug^T [d,i] : stationary = v head cols (96 + ones), moving = expS^T
                  -> row 96 = softmax denominator (colsum)
  normalize     : DVE recip of colsum row; DMA replicates it across
                  partitions (DRAM bounce, step-0 AP); all-SBUF DVE multiply
  out   [i, e]  : stationary = o_norm^T head tiles, moving = Wproj rows.
                  fp32 out, DMA to DRAM; proj+v biases added on host.

Biases for q/k/v ride as row 768 of the weight matrices against the all-ones
row 768 of xT_aug (K=1 extra contraction tile).
"""

import numpy as np

import concourse.bass as bass
import concourse.bacc as bacc
import concourse.mybir as mybir
import concourse.tile as tile

B, N, E, H = 8, 1024, 768, 8
D = E // H          # 96
DP = 128            # padded head dim (partition tile)
DA = D + 1          # 97: head dim + ones column for colsum
NT = N // 128       # 8 token tiles
ET = E // 128       # 6 embedding k-tiles
SCALE = float(E) ** -0.5

F16 = mybir.dt.float16
F32 = mybir.dt.float32
EXP = mybir.ActivationFunctionType.Exp
COPY = mybir.ActivationFunctionType.Copy


def build_program(repeats=1, loop_n=0):
    """loop_n > 0 wraps the body in a hardware For_i loop (timing use)."""
    import contextlib
    nc = bacc.Bacc("TRN2", target_bir_lowering=False)

    xT = nc.dram_tensor("xT", [E + 1, N], F16, kind="ExternalInput")
    wq = nc.dram_tensor("wq", [E + 1, H * DP], F16, kind="ExternalInput")
    wk = nc.dram_tensor("wk", [E, H * DP], F16, kind="ExternalInput")
    wv = nc.dram_tensor("wv", [E, H * DA], F16, kind="ExternalInput")
    wp = nc.dram_tensor("wp", [E, E], F16, kind="ExternalInput")
    out = nc.dram_tensor("out", [N, E], F32, kind="ExternalOutput")

    with tile.TileContext(nc) as tc:
        with (
            tc.tile_pool(name="persist", bufs=1) as persist,
            tc.tile_pool(name="exps", bufs=2) as exps,
            tc.tile_pool(name="osb", bufs=2) as osb,
            tc.tile_pool(name="outsb", bufs=2) as outp,
            tc.tile_pool(name="mmps", bufs=2, space="PSUM") as mmps,
            tc.tile_pool(name="stps", bufs=2, space="PSUM") as stps,
            tc.tile_pool(name="avps", bufs=2, space="PSUM") as avps,
            tc.tile_pool(name="dramp", bufs=2, space="DRAM") as dramp,
        ):
            loop_cm = (tc.For_i(0, loop_n, 1,
                                hint_engines=tuple(mybir.ALL_ENGINES))
                       if loop_n > 0 else contextlib.nullcontext())
            with loop_cm:
             for _rep in range(repeats):
                # ---------------- load inputs ----------------
                # DMA order = first-use order: x+wv (v phase) first, x in column
                # chunks so the first v matmuls start as soon as cols land
                x_sb, wq_sb, wk_sb, wv_sb = [], [], [], []
                for k in range(ET):
                    xk = persist.tile([128, N], F16, tag=f"x{k}", name=f"x{k}")
                    nc.sync.dma_start(out=xk, in_=xT[k * 128:(k + 1) * 128, :])
                    x_sb.append(xk)
                    vk = persist.tile([128, H * DA], F16, tag=f"wv{k}", name=f"wv{k}")
                    nc.sync.dma_start(out=vk, in_=wv[k * 128:(k + 1) * 128, :])
                    wv_sb.append(vk)
                x_last = persist.tile([1, N], F16, tag="xlast", name="x_last")
                nc.sync.dma_start(out=x_last, in_=xT[E:E + 1, :])  # all-ones row
                for k in range(ET):
                    qk = persist.tile([128, H * DP], F16, tag=f"wq{k}", name=f"wq{k}")
                    nc.sync.dma_start(out=qk, in_=wq[k * 128:(k + 1) * 128, :])
                    wq_sb.append(qk)
                    kk = persist.tile([128, H * DP], F16, tag=f"wk{k}", name=f"wk{k}")
                    nc.sync.dma_start(out=kk, in_=wk[k * 128:(k + 1) * 128, :])
                    wk_sb.append(kk)
                wq_last = persist.tile([1, H * DP], F16, tag="wqlast", name="wq_last")
                nc.sync.dma_start(out=wq_last, in_=wq[E:E + 1, :])
                wp_sb = []
                for h in range(H):
                    ph = persist.tile([D, E], F16, tag=f"wp{h}", name=f"wp{h}")
                    nc.sync.dma_start(out=ph, in_=wp[h * D:(h + 1) * D, :])
                    wp_sb.append(ph)

                # ---------------- QKV projections ----------------
                qT = [persist.tile([128, N], F16, tag=f"qT{c}", name=f"qT{c}")
                      for c in range(H)]
                kT = [persist.tile([128, N], F16, tag=f"kT{c}", name=f"kT{c}")
                      for c in range(H)]
                v_sb = [persist.tile([128, H * DA], F16, tag=f"v{n}", name=f"v{n}")
                        for n in range(NT)]

                # v first (needed by every head's AV): stationary = xT n-tile
                for n in range(NT):
                    ns = slice(n * 128, (n + 1) * 128)
                    for off, w in ((0, 512), (512, H * DA - 512)):
                        # own tag: the first v matmul must not inherit a psum-slot
                        # WAR wait on top of its DMA wait (MM allows 1 sync wait)
                        ps = mmps.tile([128, w], F32, tag="mmv", name="ps_v")
                        for k in range(ET):
                            nc.tensor.matmul(
                                ps, x_sb[k][:, ns], wv_sb[k][:, off:off + w],
                                start=(k == 0), stop=(k == ET - 1))
                        nc.vector.tensor_copy(v_sb[n][:, off:off + w], ps)
                    # ones column per head (colsum trick); softmax makes the
                    # k-bias terms cancel and the v-bias is folded on host
                    nc.vector.memset(
                        v_sb[n].rearrange("p (h a) -> p h a", h=H)[:, :, D], 1.0)

                # q carries its bias row (k/v biases cancel or fold out).
                # emit_qk_group(h, idx): one psum accumulation group (idx
                # 0/1 = q chunks, 2/3 = k chunks)
                def emit_qk_group(h, idx):
                    w_sb, w_last, dst = ((wq_sb, wq_last, qT[h]) if idx < 2
                                         else (wk_sb, None, kT[h]))
                    cs = slice(h * 128, (h + 1) * 128)
                    off = (idx % 2) * 512
                    ps = mmps.tile([128, 512], F32, tag="mm", name="ps_qkv")
                    for k in range(ET):
                        nc.tensor.matmul(
                            ps, w_sb[k][:, cs], x_sb[k][:, off:off + 512],
                            start=(k == 0),
                            stop=(k == ET - 1 and w_last is None))
                    if w_last is not None:
                        nc.tensor.matmul(
                            ps, w_last[0:1, cs], x_last[0:1, off:off + 512],
                            start=False, stop=True)
                    nc.vector.tensor_copy(dst[:, off:off + 512], ps)

                o_norm = [persist.tile([D, N], F16, tag=f"on{h}", name=f"on{h}")
                          for h in range(H)]

                def emit_av(h, ex, off):
                    # one AV chunk of head h; returns its o_sb tile slice done
                    hs = slice(h * DA, (h + 1) * DA)
                    # bufs=1 is safe now: AV chunks are emitted a head apart,
                    # so the slot's WAR on the previous o_sb copy has cleared
                    av = avps.tile([DA, 512], F32, tag="av", bufs=2,
                                   name="av_ps")
                    for j in range(NT):
                        nc.tensor.matmul(
                            av, v_sb[j][:, hs], ex[j][:, off:off + 512],
                            start=(j == 0), stop=(j == NT - 1))
                    nc.scalar.activation(o_sb[h][:, off:off + 512], av, COPY)

                def emit_norm(h, off):
                    if off == 0:
                        rcp[h] = osb.tile([1, N], F16, tag="rcp",
                                          name=f"rcp{h}")
                        with nc.allow_low_precision(reason="denom ~1e3"):
                            nc.vector.reciprocal(rcp[h], o_sb[h][D:DA, :])
                        # replicate the reciprocal row across partitions on
                        # the (idle) DMA engines instead of a PE broadcast
                        # matmul; the mul is then all-SBUF fp16 (DVE 2x mode)
                        rbc[h] = osb.tile([D, N], F16, tag="rbc",
                                          name=f"rbc{h}")
                        # SBUF APs forbid step-0 partitions; bounce the row
                        # through DRAM, whose APs allow broadcast reads
                        dr = dramp.tile([1, N], F16, tag="drcp",
                                        name=f"drcp{h}")
                        nc.sync.dma_start(out=dr[0:1, :], in_=rcp[h][0:1, :])
                        bcast = bass.AP(
                            tensor=dr.tensor, offset=dr.offset,
                            ap=[[0, D]] + [list(d) for d in dr[0:1, :].ap[1:]])
                        nc.sync.dma_start(out=rbc[h], in_=bcast)
                    nc.vector.tensor_mul(
                        o_norm[h][:, off:off + 512],
                        o_sb[h][0:D, off:off + 512],
                        rbc[h][:, off:off + 512])

                o_sb, rcp, rbc, ex_prev = {}, {}, {}, None
                for idx in range(4):
                    emit_qk_group(0, idx)
                for h in range(H):
                    # S^T+exp for head h; between j-tiles, emit next head's
                    # q/k groups and the PREVIOUS head's AV/norm — the static
                    # PE stream then always has ready matmuls after an
                    # st-slot wait (on HW exp is slower vs MMs than the
                    # scheduler's cost model assumes)
                    o_sb[h] = osb.tile([DA, N], F16, tag="osb", name=f"osb{h}")
                    ex = []
                    for j in range(NT):
                        exj = exps.tile([128, N], F16, tag=f"ex{j}", name=f"ex{h}_{j}")
                        js = slice(j * 128, (j + 1) * 128)
                        for off in (0, 512):
                            st = stps.tile([128, 512], F32, tag="st", name="st_ps")
                            nc.tensor.matmul(
                                st, kT[h][0:D, js], qT[h][0:D, off:off + 512],
                                start=True, stop=True)
                            nc.scalar.activation(
                                exj[:, off:off + 512], st, EXP, scale=SCALE)
                        ex.append(exj)
                        if h + 1 < H and j % 2 == 0:
                            emit_qk_group(h + 1, j // 2)
                        if ex_prev is not None:
                            if j == 1:
                                emit_av(h - 1, ex_prev, 0)
                            elif j == 3:
                                emit_av(h - 1, ex_prev, 512)
                            elif j == 5:
                                emit_norm(h - 1, 0)
                            elif j == 7:
                                emit_norm(h - 1, 512)
                    ex_prev = ex
                # drain the pipeline: last head's AV + norm
                emit_av(H - 1, ex_prev, 0)
                emit_av(H - 1, ex_prev, 512)
                emit_norm(H - 1, 0)
                emit_norm(H - 1, 512)

                # ---------------- output projection ----------------
                for i in range(NT):
                    isl = slice(i * 128, (i + 1) * 128)
                    for ci, (off, w) in enumerate(((0, 512), (512, E - 512))):
                        # alternate psum tags/copy engines: attention pools are
                        # idle by now, so borrow mmv slots for deeper pipelining
                        tag = "mm" if (2 * i + ci) % 2 == 0 else "mmv"
                        ps = mmps.tile([128, w], F32, tag=tag, name="ps_proj")
                        for h in range(H):
                            nc.tensor.matmul(
                                ps, o_norm[h][:, isl], wp_sb[h][:, off:off + w],
                                start=(h == 0), stop=(h == H - 1))
                        osb_t = outp.tile([128, w], F32, tag="out", name="out_sb")
                        nc.scalar.activation(osb_t, ps, COPY)
                        nc.sync.dma_start(out=out[isl, off:off + w], in_=osb_t)

    nc.compile()
    return nc


def prep_weights(Wqkv, bqkv, Wproj, bproj):
    Wr = np.asarray(Wqkv, np.float32).reshape(E, H, D, 3)
    br = np.asarray(bqkv, np.float32).reshape(H, D, 3)
    wq_full = np.zeros((E + 1, H * DP), np.float32)
    wk_full = np.zeros((E, H * DP), np.float32)
    wv_full = np.zeros((E, H * DA), np.float32)
    for h in range(H):
        wq_full[0:E, h * DP:h * DP + D] = Wr[:, h, :, 0]
        wq_full[E, h * DP:h * DP + D] = br[h, :, 0]
        wk_full[:, h * DP:h * DP + D] = Wr[:, h, :, 1]
        wv_full[:, h * DA:h * DA + D] = Wr[:, h, :, 2]
    # host-side output bias: attn rows sum to 1, so attn@(v+bv) = attn@v + bv
    # and (o + bv_cat) @ Wproj + bproj = o @ Wproj + bp_eff
    bv_cat = br[:, :, 2].reshape(E)
    bp_eff = bv_cat @ np.asarray(Wproj, np.float64) + np.asarray(bproj, np.float64)
    return {
        "wq": wq_full.astype(np.float16),
        "wk": wk_full.astype(np.float16),
        "wv": wv_full.astype(np.float16),
        "wp": np.asarray(Wproj, np.float32).astype(np.float16),
    }, bp_eff.astype(np.float32)


def make_in_maps(x, Wqkv, bqkv, Wproj, bproj):
    x = np.asarray(x, np.float32)
    shared, bp_eff = prep_weights(Wqkv, bqkv, Wproj, bproj)
    make_in_maps.bp_eff = bp_eff
    in_maps = []
    for b in range(B):
        xT_aug = np.concatenate(
            [np.ascontiguousarray(x[b].T), np.ones((1, N), np.float32)], axis=0)
        m = {"xT": xT_aug.astype(np.float16)}
        m.update(shared)
        in_maps.append(m)
    return in_maps


_prog_cache = []


def kernel(x, Wqkv, bqkv, Wproj, bproj, _run_kwargs=None):
    from concourse.bass_utils import run_bass_kernel_spmd

    in_maps = make_in_maps(x, Wqkv, bqkv, Wproj, bproj)
    if not _prog_cache:
        _prog_cache.append(build_program())
    nc = _prog_cache[0]
    res = run_bass_kernel_spmd(nc, in_maps, core_ids=list(range(B)),
                               **(_run_kwargs or {}))
    out = np.stack([r["out"] for r in res.results], axis=0)
    out = out + make_in_maps.bp_eff
    if _run_kwargs:
        kernel.last_result = res
    return out



# revision 3
# speedup vs baseline: 1.2190x; 1.2190x over previous
"""Multi-head attention Bass/Tile kernel for Trainium2, 8 cores data-parallel.

Shapes (hardcoded): x [8, 1024, 768], Wqkv [768, 2304], bqkv [2304],
Wproj [768, 768], bproj [768].  B=8 batches -> one batch per NeuronCore.

Per-core dataflow (matmul operands fp16, PSUM accumulation fp32):
  qT/kT [c, n] : stationary = W k-tiles, moving = xT.  q bias added by the
                 DVE psum->sbuf cast (tensor_scalar_add, per-partition);
                 k bias cancels in softmax, v bias folds into bp_eff on host.
  v     [n, c'] : stationary = xT n-tiles, moving = Wv (c' = 8*(96+1), with a
                 ones column per head -> AV row 96 = softmax denominator).
  S^T   [j, i] : per (head, j-tile) ONE [128,1024] psum pair; a single ACT
                 exp instruction (fused *E^-0.5) covers both 512-chunks,
                 halving ACT instruction overhead (exp is the ScalarE floor).
  o^T [97, i]  : stationary = v head cols, moving = expS^T; DVE cast to sbuf.
  normalize    : denominator rows staged to DRAM per head; ONE batched DVE
                 reciprocal on a [96,64]/[32,64] re-chopped view (the old
                 per-head [1,1024] reciprocal ran on a single DVE lane for
                 6.5us each); reciprocals bounce through DRAM for the
                 partition-broadcast read, then all-SBUF fp16 multiplies.
  out   [i, e] : stationary = o_norm head tiles, moving = Wproj rows.

Schedule: per-head j-steps emit [side-task, S^T pair, exp] so the PE always
has ready work while DVE casts drain; side tasks are the previous head's AV
groups and the next head's q/k groups (v groups during head 0). Softmax
denominators for heads 0-5 are processed in a first recip chain during head
7's scores, so only heads 6-7 gate the proj phase.
"""

import numpy as np

import concourse.bass as bass
import concourse.bacc as bacc
import concourse.mybir as mybir
import concourse.tile as tile

B, N, E, H = 8, 1024, 768, 8
D = E // H          # 96
DP = 128            # padded head dim (partition tile)
DA = D + 1          # 97: head dim + ones column for colsum
NT = N // 128       # 8 token tiles
ET = E // 128       # 6 embedding k-tiles
SCALE = float(E) ** -0.5

F16 = mybir.dt.float16
F32 = mybir.dt.float32
EXP = mybir.ActivationFunctionType.Exp


def build_program(loop_n=0):
    """loop_n > 0 wraps the body in a hardware For_i loop (timing use)."""
    import contextlib
    nc = bacc.Bacc("TRN2", target_bir_lowering=False)

    xT = nc.dram_tensor("xT", [E, N], F16, kind="ExternalInput")
    wq = nc.dram_tensor("wq", [E, H * DP], F16, kind="ExternalInput")
    wk = nc.dram_tensor("wk", [E, H * DP], F16, kind="ExternalInput")
    bq = nc.dram_tensor("bq", [DP, H], F32, kind="ExternalInput")
    wv = nc.dram_tensor("wv", [E, H * DA], F16, kind="ExternalInput")
    wp = nc.dram_tensor("wp", [E, E], F16, kind="ExternalInput")
    out = nc.dram_tensor("out", [N, E], F32, kind="ExternalOutput")

    with tile.TileContext(nc) as tc:
        with (
            tc.tile_pool(name="persist", bufs=1) as persist,
            tc.tile_pool(name="exps", bufs=2) as exps,
            tc.tile_pool(name="outsb", bufs=2) as outp,
            tc.tile_pool(name="stps", bufs=2, space="PSUM") as stps,
            tc.tile_pool(name="avps", bufs=2, space="PSUM") as avps,
            tc.tile_pool(name="mmps", bufs=2, space="PSUM") as mmps,
            tc.tile_pool(name="dramp", bufs=1, space="DRAM") as dramp,
        ):
            loop_cm = (tc.For_i(0, loop_n, 1,
                                hint_engines=tuple(mybir.ALL_ENGINES))
                       if loop_n > 0 else contextlib.nullcontext())
            with loop_cm:
                # ---------------- input DMAs, first-use order ----------------
                x_sb, wq_sb, wk_sb, wv_sb = [], [], [], []
                for k in range(ET):
                    ks = slice(k * 128, (k + 1) * 128)
                    qk = persist.tile([128, H * DP], F16, tag=f"wq{k}", name=f"wq{k}")
                    nc.sync.dma_start(out=qk, in_=wq[ks, :])
                    wq_sb.append(qk)
                    kk = persist.tile([128, H * DP], F16, tag=f"wk{k}", name=f"wk{k}")
                    nc.sync.dma_start(out=kk, in_=wk[ks, :])
                    wk_sb.append(kk)
                    xk = persist.tile([128, N], F16, tag=f"x{k}", name=f"x{k}")
                    nc.sync.dma_start(out=xk, in_=xT[ks, :])
                    x_sb.append(xk)
                bq_sb = persist.tile([DP, H], F32, tag="bq", name="bq_sb")
                nc.sync.dma_start(out=bq_sb, in_=bq[:, :])
                for k in range(ET):
                    ks = slice(k * 128, (k + 1) * 128)
                    vk = persist.tile([128, H * DA], F16, tag=f"wv{k}", name=f"wv{k}")
                    nc.sync.dma_start(out=vk, in_=wv[ks, :])
                    wv_sb.append(vk)
                wp_sb = []
                for h in range(H):
                    ph = persist.tile([D, E], F16, tag=f"wp{h}", name=f"wp{h}")
                    nc.sync.dma_start(out=ph, in_=wp[h * D:(h + 1) * D, :])
                    wp_sb.append(ph)

                # ---------------- persistent sbuf tiles ----------------
                qT = [persist.tile([128, N], F16, tag=f"qT{c}", name=f"qT{c}")
                      for c in range(H)]
                kT = [persist.tile([128, N], F16, tag=f"kT{c}", name=f"kT{c}")
                      for c in range(H)]
                v_sb = [persist.tile([128, H * DA], F16, tag=f"v{n}", name=f"v{n}")
                        for n in range(NT)]
                o_sb = [persist.tile([DA, N], F16, tag=f"o{h}", name=f"o{h}")
                        for h in range(H)]
                o_norm = [persist.tile([D, N], F16, tag=f"on{h}", name=f"on{h}")
                          for h in range(H)]
                rbc = [persist.tile([D, N], F16, tag=f"rbc{h}", name=f"rbc{h}")
                       for h in range(H)]
                dn_dram = dramp.tile([H, N], F16, tag="dn", name="dn_dram")
                rc_dram = dramp.tile([H, N], F16, tag="rc", name="rc_dram")

                def emit_qk_group(h, idx):
                    # idx 0/1 = q off-chunks, 2/3 = k off-chunks
                    is_q = idx < 2
                    off = (idx % 2) * 512
                    w_sb, dst = (wq_sb, qT[h]) if is_q else (wk_sb, kT[h])
                    cs = slice(h * DP, (h + 1) * DP)
                    ps = mmps.tile([128, 512], F32, tag="mm", name="ps_qk")
                    for k in range(ET):
                        nc.tensor.matmul(ps, w_sb[k][:, cs],
                                         x_sb[k][:, off:off + 512],
                                         start=(k == 0), stop=(k == ET - 1))
                    if is_q:
                        nc.vector.tensor_scalar_add(dst[:, off:off + 512], ps,
                                                    bq_sb[:, h:h + 1])
                    else:
                        nc.vector.tensor_copy(dst[:, off:off + 512], ps)

                def emit_v_group(n):
                    ns = slice(n * 128, (n + 1) * 128)
                    for off, w in ((0, 512), (512, H * DA - 512)):
                        ps = mmps.tile([128, w], F32, tag="mm", name="ps_v")
                        for k in range(ET):
                            nc.tensor.matmul(ps, x_sb[k][:, ns],
                                             wv_sb[k][:, off:off + w],
                                             start=(k == 0), stop=(k == ET - 1))
                        nc.vector.tensor_copy(v_sb[n][:, off:off + w], ps)
                    nc.vector.memset(
                        v_sb[n].rearrange("p (h a) -> p h a", h=H)[:, :, D], 1.0)

                def emit_av_group(h, ex, off):
                    hs = slice(h * DA, (h + 1) * DA)
                    av = avps.tile([DA, 512], F32, tag="av", name="av_ps")
                    for j in range(NT):
                        nc.tensor.matmul(av, v_sb[j][:, hs],
                                         ex[j][:, off:off + 512],
                                         start=(j == 0), stop=(j == NT - 1))
                    nc.vector.tensor_copy(o_sb[h][:, off:off + 512], av)
                    if off == 512:
                        # full denominator row now in sbuf; stage to DRAM
                        nc.sync.dma_start(out=dn_dram[h:h + 1, :],
                                          in_=o_sb[h][D:DA, :])

                def emit_recip_chain(h0, nh):
                    # gather denom rows h0..h0+nh-1 into [16*nh, 64], one DVE
                    # reciprocal, bounce back to DRAM for the broadcast reads
                    p = nh * (N // 64)  # 16 partitions per head
                    dn16 = persist.tile([p, 64], F16, tag=f"dn16_{h0}",
                                        name=f"dn16_{h0}")
                    nc.sync.dma_start(
                        out=dn16,
                        in_=dn_dram[h0:h0 + nh, :].rearrange(
                            "h (p c) -> (h p) c", c=64))
                    rcp16 = persist.tile([p, 64], F16, tag=f"rcp16_{h0}",
                                         name=f"rcp16_{h0}")
                    with nc.allow_low_precision(reason="denom ~1e3"):
                        nc.vector.reciprocal(rcp16, dn16)
                    nc.sync.dma_start(
                        out=rc_dram[h0:h0 + nh, :].rearrange(
                            "h (p c) -> (h p) c", c=64),
                        in_=rcp16)
                    for h in range(h0, h0 + nh):
                        row = rc_dram[h:h + 1, :]
                        bc = bass.AP(tensor=row.tensor, offset=row.offset,
                                     ap=[[0, D]] + [list(d) for d in row.ap[1:]])
                        nc.sync.dma_start(out=rbc[h], in_=bc)

                def emit_norm(h):
                    for off in (0, 512):
                        nc.vector.tensor_mul(o_norm[h][:, off:off + 512],
                                             o_sb[h][0:D, off:off + 512],
                                             rbc[h][:, off:off + 512])

                def emit_proj_group(i, ci):
                    isl = slice(i * 128, (i + 1) * 128)
                    off, w = ((0, 512), (512, E - 512))[ci]
                    ps = mmps.tile([128, w], F32, tag="mm", name="ps_proj")
                    for h in range(H):
                        nc.tensor.matmul(ps, o_norm[h][:, isl],
                                         wp_sb[h][:, off:off + w],
                                         start=(h == 0), stop=(h == H - 1))
                    ot = outp.tile([128, w], F32, tag="out", name="out_sb")
                    nc.vector.tensor_copy(ot, ps)
                    nc.sync.dma_start(out=out[isl, off:off + w], in_=ot)

                # ---------------- main schedule ----------------
                for idx in range(4):
                    emit_qk_group(0, idx)

                ex_prev = None
                for h in range(H):
                    ex = []
                    for j in range(NT):
                        # side task first: keeps ready PE work ahead of the
                        # S^T matmuls, which wait on the previous casts
                        if h == 0:
                            emit_v_group(j)
                            if j >= 4:
                                emit_qk_group(1, j - 4)
                        else:
                            if j == 0:
                                emit_av_group(h - 1, ex_prev, 0)
                            elif j == 2:
                                emit_av_group(h - 1, ex_prev, 512)
                            elif h < H - 1 and j in (1, 3, 4, 5):
                                emit_qk_group(h + 1, {1: 0, 3: 1, 4: 2, 5: 3}[j])
                        js = slice(j * 128, (j + 1) * 128)
                        st = stps.tile([128, N], F32, tag="st", name="st_ps")
                        nc.tensor.matmul(st[:, 0:512], kT[h][0:D, js],
                                         qT[h][0:D, 0:512],
                                         start=True, stop=True)
                        nc.tensor.matmul(st[:, 512:1024], kT[h][0:D, js],
                                         qT[h][0:D, 512:1024],
                                         start=True, stop=True)
                        exj = exps.tile([128, N], F16, tag=f"ex{j}",
                                        name=f"ex{h}_{j}")
                        nc.scalar.activation(exj, st, EXP, scale=SCALE)
                        ex.append(exj)
                    if h == 6:
                        emit_recip_chain(0, 6)
                    if h == 7:
                        for hh in range(6):
                            emit_norm(hh)
                    ex_prev = ex

                # drain: last head's AV, short recip chain, last norms, proj
                emit_av_group(7, ex_prev, 0)
                emit_av_group(7, ex_prev, 512)
                emit_recip_chain(6, 2)
                emit_norm(6)
                emit_norm(7)
                for i in range(NT):
                    for ci in (0, 1):
                        emit_proj_group(i, ci)

    nc.compile()
    return nc


def prep_weights(Wqkv, bqkv, Wproj, bproj):
    Wr = np.asarray(Wqkv, np.float32).reshape(E, H, D, 3)
    br = np.asarray(bqkv, np.float32).reshape(H, D, 3)
    wq_full = np.zeros((E, H * DP), np.float32)
    wk_full = np.zeros((E, H * DP), np.float32)
    wv_full = np.zeros((E, H * DA), np.float32)
    bq_full = np.zeros((DP, H), np.float32)
    for h in range(H):
        wq_full[:, h * DP:h * DP + D] = Wr[:, h, :, 0]
        wk_full[:, h * DP:h * DP + D] = Wr[:, h, :, 1]
        wv_full[:, h * DA:h * DA + D] = Wr[:, h, :, 2]
        bq_full[0:D, h] = br[h, :, 0]
    # host-side output bias: attn rows sum to 1, so attn@(v+bv) = attn@v + bv
    # and (o + bv_cat) @ Wproj + bproj = o @ Wproj + bp_eff
    bv_cat = br[:, :, 2].reshape(E)
    bp_eff = bv_cat @ np.asarray(Wproj, np.float64) + np.asarray(bproj, np.float64)
    return {
        "wq": wq_full.astype(np.float16),
        "wk": wk_full.astype(np.float16),
        "wv": wv_full.astype(np.float16),
        "wp": np.asarray(Wproj, np.float32).astype(np.float16),
        "bq": bq_full,
    }, bp_eff.astype(np.float32)


def make_in_maps(x, Wqkv, bqkv, Wproj, bproj):
    x = np.asarray(x, np.float32)
    shared, bp_eff = prep_weights(Wqkv, bqkv, Wproj, bproj)
    make_in_maps.bp_eff = bp_eff
    in_maps = []
    for b in range(B):
        m = {"xT": np.ascontiguousarray(x[b].T).astype(np.float16)}
        m.update(shared)
        in_maps.append(m)
    return in_maps


_prog_cache = []


def kernel(x, Wqkv, bqkv, Wproj, bproj, _run_kwargs=None):
    from concourse.bass_utils import run_bass_kernel_spmd

    in_maps = make_in_maps(x, Wqkv, bqkv, Wproj, bproj)
    if not _prog_cache:
        _prog_cache.append(build_program())
    nc = _prog_cache[0]
    res = run_bass_kernel_spmd(nc, in_maps, core_ids=list(range(B)),
                               **(_run_kwargs or {}))
    out = np.stack([r["out"] for r in res.results], axis=0)
    out = out + make_in_maps.bp_eff
    if _run_kwargs:
        kernel.last_result = res
    return out


# revision 10
# speedup vs baseline: 1.2541x; 1.0288x over previous
"""Multi-head attention Bass/Tile kernel for Trainium2, 8 cores data-parallel.

Shapes (hardcoded): x [8, 1024, 768], Wqkv [768, 2304], bqkv [2304],
Wproj [768, 768], bproj [768].  B=8 batches -> one batch per NeuronCore.

Per-core dataflow (matmul operands fp16, PSUM accumulation fp32):
  qT/kT [c, n] : stationary = W k-tiles, moving = xT.  q bias added by the
                 DVE psum->sbuf cast (tensor_scalar_add, per-partition);
                 k bias cancels in softmax, v bias folds into bp_eff on host.
  v     [n, c'] : stationary = xT n-tiles, moving = Wv (c' = 8*(96+1), with a
                 ones column per head -> AV row 96 = softmax denominator).
  S^T   [j, i] : per (head, j-tile) ONE [128,1024] psum pair; a single ACT
                 exp instruction (fused *E^-0.5) covers both 512-chunks,
                 halving ACT instruction overhead (exp is the ScalarE floor).
  o^T [97, i]  : stationary = v head cols, moving = expS^T; DVE cast to sbuf.
  normalize    : denominator rows staged to DRAM per head; ONE batched DVE
                 reciprocal on a [96,64]/[32,64] re-chopped view (the old
                 per-head [1,1024] reciprocal ran on a single DVE lane for
                 6.5us each); reciprocals bounce through DRAM for the
                 partition-broadcast read, then all-SBUF fp16 multiplies.
  out   [i, e] : stationary = o_norm head tiles, moving = Wproj rows.

Schedule: per-head j-steps emit [side-task, S^T pair, exp] so the PE always
has ready work while DVE casts drain; side tasks are the previous head's AV
groups and the next head's q/k groups (v groups during head 0). Softmax
denominators for heads 0-5 are processed in a first recip chain during head
7's scores, so only heads 6-7 gate the proj phase.
"""

import numpy as np

import concourse.bass as bass
import concourse.bacc as bacc
import concourse.mybir as mybir
import concourse.tile as tile

B, N, E, H = 8, 1024, 768, 8
D = E // H          # 96
DP = 128            # padded head dim (partition tile)
DA = D + 1          # 97: head dim + ones column for colsum
NT = N // 128       # 8 token tiles
ET = E // 128       # 6 embedding k-tiles
SCALE = float(E) ** -0.5

F16 = mybir.dt.float16
F32 = mybir.dt.float32
EXP = mybir.ActivationFunctionType.Exp


def build_program(loop_n=0):
    """loop_n > 0 wraps the body in a hardware For_i loop (timing use)."""
    import contextlib
    nc = bacc.Bacc("TRN2", target_bir_lowering=False)

    xT = nc.dram_tensor("xT", [E, N], F16, kind="ExternalInput")
    wq = nc.dram_tensor("wq", [E, H * DP], F16, kind="ExternalInput")
    wk = nc.dram_tensor("wk", [E, H * DP], F16, kind="ExternalInput")
    bq = nc.dram_tensor("bq", [DP, H], F32, kind="ExternalInput")
    wv = nc.dram_tensor("wv", [E, H * DA], F16, kind="ExternalInput")
    wp = nc.dram_tensor("wp", [E, E], F16, kind="ExternalInput")
    out = nc.dram_tensor("out", [N, E], F16, kind="ExternalOutput")

    with tile.TileContext(nc) as tc:
        with (
            tc.tile_pool(name="persist", bufs=1) as persist,
            tc.tile_pool(name="exps", bufs=2) as exps,
            tc.tile_pool(name="outsb", bufs=2) as outp,
            tc.tile_pool(name="stps", bufs=2, space="PSUM") as stps,
            tc.tile_pool(name="avps", bufs=2, space="PSUM") as avps,
            tc.tile_pool(name="mmps", bufs=2, space="PSUM") as mmps,
            tc.tile_pool(name="dramp", bufs=1, space="DRAM") as dramp,
        ):
            loop_cm = (tc.For_i(0, loop_n, 1,
                                hint_engines=tuple(mybir.ALL_ENGINES))
                       if loop_n > 0 else contextlib.nullcontext())
            with loop_cm:
                # ---------------- input DMAs, first-use order ----------------
                x_sb, wq_sb, wk_sb, wv_sb = [], [], [], []
                for k in range(ET):
                    ks = slice(k * 128, (k + 1) * 128)
                    qk = persist.tile([128, H * DP], F16, tag=f"wq{k}", name=f"wq{k}")
                    nc.sync.dma_start(out=qk, in_=wq[ks, :])
                    wq_sb.append(qk)
                    kk = persist.tile([128, H * DP], F16, tag=f"wk{k}", name=f"wk{k}")
                    nc.sync.dma_start(out=kk, in_=wk[ks, :])
                    wk_sb.append(kk)
                    xk = persist.tile([128, N], F16, tag=f"x{k}", name=f"x{k}")
                    nc.sync.dma_start(out=xk, in_=xT[ks, :])
                    x_sb.append(xk)
                bq_sb = persist.tile([DP, H], F32, tag="bq", name="bq_sb")
                nc.sync.dma_start(out=bq_sb, in_=bq[:, :])
                for k in range(ET):
                    ks = slice(k * 128, (k + 1) * 128)
                    vk = persist.tile([128, H * DA], F16, tag=f"wv{k}", name=f"wv{k}")
                    nc.sync.dma_start(out=vk, in_=wv[ks, :])
                    wv_sb.append(vk)
                wp_sb = []
                for h in range(H):
                    ph = persist.tile([D, E], F16, tag=f"wp{h}", name=f"wp{h}")
                    nc.sync.dma_start(out=ph, in_=wp[h * D:(h + 1) * D, :])
                    wp_sb.append(ph)

                # ---------------- persistent sbuf tiles ----------------
                qT = [persist.tile([128, N], F16, tag=f"qT{c}", name=f"qT{c}")
                      for c in range(H)]
                kT = [persist.tile([128, N], F16, tag=f"kT{c}", name=f"kT{c}")
                      for c in range(H)]
                v_sb = [persist.tile([128, H * DA], F16, tag=f"v{n}", name=f"v{n}")
                        for n in range(NT)]
                o_sb = [persist.tile([DA, N], F16, tag=f"o{h}", name=f"o{h}")
                        for h in range(H)]
                o_norm = [persist.tile([D, N], F16, tag=f"on{h}", name=f"on{h}")
                          for h in range(H)]
                rbc = [persist.tile([D, N], F16, tag=f"rbc{h}", name=f"rbc{h}")
                       for h in range(H)]
                # denominators: 16 partitions x 64 per head (1024 chopped);
                # head 7 gets its own tile (engine base partition must be
                # 32-aligned, so rows 112:128 can't start a DVE op)
                dn16 = persist.tile([112, 64], F16, tag="dn16", name="dn16")
                rcp16 = persist.tile([112, 64], F16, tag="rcp16", name="rcp16")
                dn16b = persist.tile([16, 64], F16, tag="dn16b", name="dn16b")
                rcp16b = persist.tile([16, 64], F16, tag="rcp16b", name="rcp16b")
                rc_dram = dramp.tile([H, N], F16, tag="rc", name="rc_dram")

                def emit_qk_group(h, idx):
                    # idx 0/1 = q off-chunks, 2/3 = k off-chunks
                    is_q = idx < 2
                    off = (idx % 2) * 512
                    w_sb, dst = (wq_sb, qT[h]) if is_q else (wk_sb, kT[h])
                    cs = slice(h * DP, (h + 1) * DP)
                    ps = mmps.tile([128, 512], F32, tag="mm", name="ps_qk")
                    for k in range(ET):
                        nc.tensor.matmul(ps, w_sb[k][:, cs],
                                         x_sb[k][:, off:off + 512],
                                         start=(k == 0), stop=(k == ET - 1))
                    if is_q:
                        nc.vector.tensor_scalar_add(dst[:, off:off + 512], ps,
                                                    bq_sb[:, h:h + 1])
                    else:
                        nc.vector.tensor_copy(dst[:, off:off + 512], ps)

                def emit_v_group(n):
                    ns = slice(n * 128, (n + 1) * 128)
                    for off, w in ((0, 512), (512, H * DA - 512)):
                        ps = mmps.tile([128, w], F32, tag="mm", name="ps_v")
                        for k in range(ET):
                            nc.tensor.matmul(ps, x_sb[k][:, ns],
                                             wv_sb[k][:, off:off + w],
                                             start=(k == 0), stop=(k == ET - 1))
                        nc.vector.tensor_copy(v_sb[n][:, off:off + w], ps)
                    nc.vector.memset(
                        v_sb[n].rearrange("p (h a) -> p h a", h=H)[:, :, D], 1.0)

                def emit_av_group(h, ex, off):
                    hs = slice(h * DA, (h + 1) * DA)
                    av = avps.tile([DA, 512], F32, tag="av", name="av_ps")
                    for j in range(NT):
                        nc.tensor.matmul(av, v_sb[j][:, hs],
                                         ex[j][:, off:off + 512],
                                         start=(j == 0), stop=(j == NT - 1))
                    nc.vector.tensor_copy(o_sb[h][:, off:off + 512], av)
                    if off == 512:
                        # full denominator row in sbuf; chop straight into the
                        # [16,64] partition-parallel slot (sbuf->sbuf DMA)
                        dst = (dn16[h * 16:(h + 1) * 16, :] if h < 7
                               else dn16b[:, :])
                        nc.sync.dma_start(out=dst, in_=o_sb[h][D:DA, :])

                def emit_recip_chain(h0, nh):
                    # one batched DVE reciprocal over heads h0..h0+nh-1, then
                    # bounce through DRAM for the partition-broadcast reads
                    src, dst = ((dn16[h0 * 16:(h0 + nh) * 16, :],
                                 rcp16[h0 * 16:(h0 + nh) * 16, :])
                                if h0 < 7 else (dn16b[:, :], rcp16b[:, :]))
                    with nc.allow_low_precision(reason="denom ~1e3"):
                        nc.vector.reciprocal(dst, src)
                    nc.sync.dma_start(
                        out=rc_dram[h0:h0 + nh, :].rearrange(
                            "h (p c) -> (h p) c", c=64),
                        in_=dst)
                    for h in range(h0, h0 + nh):
                        row = rc_dram[h:h + 1, :]
                        bc = bass.AP(tensor=row.tensor, offset=row.offset,
                                     ap=[[0, D]] + [list(d) for d in row.ap[1:]])
                        nc.sync.dma_start(out=rbc[h], in_=bc)

                def emit_norm(h):
                    for off in (0, 512):
                        nc.vector.tensor_mul(o_norm[h][:, off:off + 512],
                                             o_sb[h][0:D, off:off + 512],
                                             rbc[h][:, off:off + 512])

                def proj_ps(g, w):
                    pool = mmps if g % 2 == 0 else stps
                    return pool.tile([128, w], F32, tag="mm" if g % 2 == 0
                                     else "st", name="ps_proj")

                def emit_proj_mms(ps, i, ci, hs):
                    isl = slice(i * 128, (i + 1) * 128)
                    off, w = ((0, 512), (512, E - 512))[ci]
                    for h in hs:
                        nc.tensor.matmul(ps, o_norm[h][:, isl],
                                         wp_sb[h][:, off:off + w],
                                         start=(h == 0), stop=(h == H - 1))

                def emit_proj_fin(ps, i, ci):
                    isl = slice(i * 128, (i + 1) * 128)
                    off, w = ((0, 512), (512, E - 512))[ci]
                    ot = outp.tile([128, w], F16, tag="out", name="out_sb")
                    nc.vector.tensor_copy(ot, ps)
                    nc.sync.dma_start(out=out[isl, off:off + w], in_=ot)

                # ---------------- main schedule ----------------
                for idx in range(4):
                    emit_qk_group(0, idx)

                ex_prev = None
                for h in range(H):
                    ex = []
                    for j in range(NT):
                        # side task first: keeps ready PE work ahead of the
                        # S^T matmuls, which wait on the previous casts
                        if h == 0:
                            emit_v_group(j)
                            if j in (3, 4, 5, 6):
                                emit_qk_group(1, j - 3)
                        else:
                            if j == 0:
                                emit_av_group(h - 1, ex_prev, 0)
                            elif j == 2:
                                emit_av_group(h - 1, ex_prev, 512)
                            elif h < H - 1 and j in (1, 3, 4, 5):
                                emit_qk_group(h + 1, {1: 0, 3: 1, 4: 2, 5: 3}[j])
                        js = slice(j * 128, (j + 1) * 128)
                        st = stps.tile([128, N], F32, tag="st", name="st_ps")
                        nc.tensor.matmul(st[:, 0:512], kT[h][0:D, js],
                                         qT[h][0:D, 0:512],
                                         start=True, stop=True)
                        nc.tensor.matmul(st[:, 512:1024], kT[h][0:D, js],
                                         qT[h][0:D, 512:1024],
                                         start=True, stop=True)
                        exj = exps.tile([128, N], F16, tag=f"ex{j}",
                                        name=f"ex{h}_{j}")
                        nc.scalar.activation(exj, st, EXP, scale=SCALE)
                        ex.append(exj)
                        if h == 7 and j == 3:
                            # heads 0-6 denominators are all staged by now
                            emit_recip_chain(0, 7)
                            for hh in range(7):
                                emit_norm(hh)
                    ex_prev = ex

                # drain: last head's AV, its short recip chain overlapped with
                # the first proj groups' head-0..6 accumulation passes
                emit_av_group(7, ex_prev, 0)
                emit_av_group(7, ex_prev, 512)
                emit_recip_chain(7, 1)
                pre = []
                for g in range(4):
                    i, ci = g // 2, g % 2
                    ps = proj_ps(g, ((0, 512), (512, E - 512))[ci][1])
                    emit_proj_mms(ps, i, ci, range(7))
                    pre.append((ps, i, ci))
                emit_norm(7)
                for ps, i, ci in pre:
                    emit_proj_mms(ps, i, ci, [7])
                    emit_proj_fin(ps, i, ci)
                for g in range(4, 2 * NT):
                    i, ci = g // 2, g % 2
                    ps = proj_ps(g, ((0, 512), (512, E - 512))[ci][1])
                    emit_proj_mms(ps, i, ci, range(H))
                    emit_proj_fin(ps, i, ci)

    nc.compile()
    return nc


def prep_weights(Wqkv, bqkv, Wproj, bproj):
    Wr = np.asarray(Wqkv, np.float32).reshape(E, H, D, 3)
    br = np.asarray(bqkv, np.float32).reshape(H, D, 3)
    wq_full = np.zeros((E, H * DP), np.float32)
    wk_full = np.zeros((E, H * DP), np.float32)
    wv_full = np.zeros((E, H * DA), np.float32)
    bq_full = np.zeros((DP, H), np.float32)
    for h in range(H):
        wq_full[:, h * DP:h * DP + D] = Wr[:, h, :, 0]
        wk_full[:, h * DP:h * DP + D] = Wr[:, h, :, 1]
        wv_full[:, h * DA:h * DA + D] = Wr[:, h, :, 2]
        bq_full[0:D, h] = br[h, :, 0]
    # host-side output bias: attn rows sum to 1, so attn@(v+bv) = attn@v + bv
    # and (o + bv_cat) @ Wproj + bproj = o @ Wproj + bp_eff
    bv_cat = br[:, :, 2].reshape(E)
    bp_eff = bv_cat @ np.asarray(Wproj, np.float64) + np.asarray(bproj, np.float64)
    return {
        "wq": wq_full.astype(np.float16),
        "wk": wk_full.astype(np.float16),
        "wv": wv_full.astype(np.float16),
        "wp": np.asarray(Wproj, np.float32).astype(np.float16),
        "bq": bq_full,
    }, bp_eff.astype(np.float32)


def make_in_maps(x, Wqkv, bqkv, Wproj, bproj):
    x = np.asarray(x, np.float32)
    shared, bp_eff = prep_weights(Wqkv, bqkv, Wproj, bproj)
    make_in_maps.bp_eff = bp_eff
    in_maps = []
    for b in range(B):
        m = {"xT": np.ascontiguousarray(x[b].T).astype(np.float16)}
        m.update(shared)
        in_maps.append(m)
    return in_maps


_prog_cache = []


def kernel(x, Wqkv, bqkv, Wproj, bproj, _run_kwargs=None):
    from concourse.bass_utils import run_bass_kernel_spmd

    in_maps = make_in_maps(x, Wqkv, bqkv, Wproj, bproj)
    if not _prog_cache:
        _prog_cache.append(build_program())
    nc = _prog_cache[0]
    res = run_bass_kernel_spmd(nc, in_maps, core_ids=list(range(B)),
                               **(_run_kwargs or {}))
    out = np.stack([r["out"] for r in res.results], axis=0)
    out = out + make_in_maps.bp_eff
    if _run_kwargs:
        kernel.last_result = res
    return out


# revision 21
# speedup vs baseline: 1.4394x; 1.1478x over previous
"""Multi-head attention Bass/Tile kernel for Trainium2, 8 cores data-parallel.

Shapes (hardcoded): x [8, 1024, 768], Wqkv [768, 2304], bqkv [2304],
Wproj [768, 768], bproj [768].  B=8 batches -> one batch per NeuronCore.

Per-core dataflow (matmul operands fp16, PSUM accumulation fp32):
  qT/kT [c, n] : stationary = W k-tiles, moving = xT.  q bias added by the
                 DVE psum->sbuf cast (tensor_scalar_add, per-partition);
                 k bias cancels in softmax, v bias folds into bp_eff on host.
  v     [n, c'] : stationary = xT n-tiles, moving = Wv (c' = 8*(96+1), with a
                 ones column per head -> AV row 96 = softmax denominator).
  S^T   [j, i] : per (head, j-tile) ONE [128,1024] psum pair; a single ACT
                 exp instruction (fused *E^-0.5) covers both 512-chunks,
                 halving ACT instruction overhead (exp is the ScalarE floor).
  o^T [97, i]  : stationary = v head cols, moving = expS^T; DVE cast to sbuf.
  normalize    : denominator rows staged to DRAM per head; ONE batched DVE
                 reciprocal on a [96,64]/[32,64] re-chopped view (the old
                 per-head [1,1024] reciprocal ran on a single DVE lane for
                 6.5us each); reciprocals bounce through DRAM for the
                 partition-broadcast read, then all-SBUF fp16 multiplies.
  out   [i, e] : stationary = o_norm head tiles, moving = Wproj rows.

Schedule: per-head j-steps emit [side-task, S^T pair, exp] so the PE always
has ready work while DVE casts drain; side tasks are the previous head's AV
groups and the next head's q/k groups (v groups during head 0). Softmax
denominators for heads 0-5 are processed in a first recip chain during head
7's scores, so only heads 6-7 gate the proj phase.
"""

import numpy as np

import concourse.bass as bass
import concourse.bacc as bacc
import concourse.mybir as mybir
import concourse.tile as tile

B, N, E, H = 8, 1024, 768, 8
D = E // H          # 96
DP = 128            # padded head dim (partition tile)
DA = D + 1          # 97: head dim + ones column for colsum
NT = N // 128       # 8 token tiles
ET = E // 128       # 6 embedding k-tiles
SCALE = float(E) ** -0.5

F16 = mybir.dt.float16
F32 = mybir.dt.float32
F8 = mybir.dt.float8e4
EXP = mybir.ActivationFunctionType.Exp
DR = mybir.MatmulPerfMode.DoubleRow


def build_program(loop_n=0):
    """loop_n > 0 wraps the body in a hardware For_i loop (timing use)."""
    import contextlib
    nc = bacc.Bacc("TRN2", target_bir_lowering=False)

    xT = nc.dram_tensor("xT", [E, N], F16, kind="ExternalInput")
    # fp8 copies for the q/k path, pre-chopped [partition, k-chunk, col] so
    # DoubleRow can contract 256 rows per pass (k-chunk pairs)
    x8 = nc.dram_tensor("x8", [128, ET, N], F8, kind="ExternalInput")
    wq8 = nc.dram_tensor("wq8", [128, ET, H * DP], F8, kind="ExternalInput")
    wk8 = nc.dram_tensor("wk8", [128, ET, H * DP], F8, kind="ExternalInput")
    bq = nc.dram_tensor("bq", [DP, H], F32, kind="ExternalInput")
    wv = nc.dram_tensor("wv", [E, H * DA], F16, kind="ExternalInput")
    wp = nc.dram_tensor("wp", [E, E], F16, kind="ExternalInput")
    out = nc.dram_tensor("out", [N, E], F16, kind="ExternalOutput")

    with tile.TileContext(nc) as tc:
        with (
            tc.tile_pool(name="persist", bufs=1) as persist,
            tc.tile_pool(name="exps", bufs=2) as exps,
            tc.tile_pool(name="outsb", bufs=2) as outp,
            tc.tile_pool(name="stps", bufs=2, space="PSUM") as stps,
            tc.tile_pool(name="avps", bufs=2, space="PSUM") as avps,
            tc.tile_pool(name="mmps", bufs=2, space="PSUM") as mmps,
            tc.tile_pool(name="dramp", bufs=1, space="DRAM") as dramp,
        ):
            loop_cm = (tc.For_i(0, loop_n, 1,
                                hint_engines=tuple(mybir.ALL_ENGINES))
                       if loop_n > 0 else contextlib.nullcontext())
            with loop_cm:
                # ---------------- input DMAs, first-use order ----------------
                x8_sb = persist.tile([128, ET, N], F8, tag="x8", name="x8_sb")
                wq8_sb = persist.tile([128, ET, H * DP], F8, tag="wq8",
                                      name="wq8_sb")
                wk8_sb = persist.tile([128, ET, H * DP], F8, tag="wk8",
                                      name="wk8_sb")
                for c in range(ET):
                    nc.sync.dma_start(out=x8_sb[:, c, :], in_=x8[:, c, :])
                    nc.sync.dma_start(out=wq8_sb[:, c, :], in_=wq8[:, c, :])
                    nc.sync.dma_start(out=wk8_sb[:, c, :], in_=wk8[:, c, :])
                bq_sb = persist.tile([DP, H], F32, tag="bq", name="bq_sb")
                nc.sync.dma_start(out=bq_sb, in_=bq[:, :])
                x_sb, wv_sb = [], []
                for k in range(ET):
                    ks = slice(k * 128, (k + 1) * 128)
                    xk = persist.tile([128, N], F16, tag=f"x{k}", name=f"x{k}")
                    nc.sync.dma_start(out=xk, in_=xT[ks, :])
                    x_sb.append(xk)
                    vk = persist.tile([128, H * DA], F16, tag=f"wv{k}", name=f"wv{k}")
                    nc.sync.dma_start(out=vk, in_=wv[ks, :])
                    wv_sb.append(vk)
                wp_sb = []
                for h in range(H):
                    ph = persist.tile([D, E], F16, tag=f"wp{h}", name=f"wp{h}")
                    nc.sync.dma_start(out=ph, in_=wp[h * D:(h + 1) * D, :])
                    wp_sb.append(ph)

                # ---------------- persistent sbuf tiles ----------------
                qT = [persist.tile([128, N], F8, tag=f"qT{c}", name=f"qT{c}")
                      for c in range(H)]
                kT = [persist.tile([128, N], F8, tag=f"kT{c}", name=f"kT{c}")
                      for c in range(H)]
                v_sb = [persist.tile([128, H * DA], F16, tag=f"v{n}", name=f"v{n}")
                        for n in range(NT)]
                o_sb = [persist.tile([DA, N], F16, tag=f"o{h}", name=f"o{h}")
                        for h in range(H)]
                o_norm = [persist.tile([D, N], F16, tag=f"on{h}", name=f"on{h}")
                          for h in range(H)]
                rbc = [persist.tile([D, N], F16, tag=f"rbc{h}", name=f"rbc{h}")
                       for h in range(H - 1)]
                ones1 = persist.tile([1, D], F16, tag="ones1", name="ones1")
                r7row = persist.tile([1, N], F16, tag="r7row", name="r7row")
                nc.vector.memset(ones1, 1.0)
                # denominators: 16 partitions x 64 per head (1024 chopped);
                # head 7 gets its own tile (engine base partition must be
                # 32-aligned, so rows 112:128 can't start a DVE op)
                dn16 = persist.tile([112, 64], F16, tag="dn16", name="dn16")
                rcp16 = persist.tile([112, 64], F16, tag="rcp16", name="rcp16")
                dn16b = persist.tile([16, 64], F16, tag="dn16b", name="dn16b")
                rcp16b = persist.tile([16, 64], F16, tag="rcp16b", name="rcp16b")
                rc_dram = dramp.tile([H, N], F16, tag="rc", name="rc_dram")

                def emit_qk_group(h, idx):
                    # idx 0/1 = q off-chunks, 2/3 = k off-chunks. fp8
                    # DoubleRow: 256 contraction rows per pass (3 passes).
                    is_q = idx < 2
                    off = (idx % 2) * 512
                    w8, dst = (wq8_sb, qT[h]) if is_q else (wk8_sb, kT[h])
                    cs = slice(h * DP, (h + 1) * DP)
                    ps = mmps.tile([128, 512], F32, tag="mm", name="ps_qk")
                    for t in range(ET // 2):
                        nc.tensor.matmul(ps, w8[:, 2 * t:2 * t + 2, cs],
                                         x8_sb[:, 2 * t:2 * t + 2, off:off + 512],
                                         start=(t == 0), stop=(t == ET // 2 - 1),
                                         perf_mode=DR)
                    if is_q:
                        nc.vector.tensor_scalar_add(dst[:, off:off + 512], ps,
                                                    bq_sb[:, h:h + 1])
                    else:
                        nc.vector.tensor_copy(dst[:, off:off + 512], ps)

                def emit_v_group(n):
                    ns = slice(n * 128, (n + 1) * 128)
                    for off, w in ((0, 512), (512, H * DA - 512)):
                        ps = mmps.tile([128, w], F32, tag="mm", name="ps_v")
                        for k in range(ET):
                            nc.tensor.matmul(ps, x_sb[k][:, ns],
                                             wv_sb[k][:, off:off + w],
                                             start=(k == 0), stop=(k == ET - 1))
                        nc.vector.tensor_copy(v_sb[n][:, off:off + w], ps)
                    nc.vector.memset(
                        v_sb[n].rearrange("p (h a) -> p h a", h=H)[:, :, D], 1.0)

                def emit_av_group(h, ex, off):
                    hs = slice(h * DA, (h + 1) * DA)
                    av = avps.tile([DA, 512], F32, tag="av", name="av_ps")
                    for j in range(NT):
                        nc.tensor.matmul(av, v_sb[j][:, hs],
                                         ex[j][:, off:off + 512],
                                         start=(j == 0), stop=(j == NT - 1))
                    nc.vector.tensor_copy(o_sb[h][:, off:off + 512], av)
                    if off == 512:
                        # full denominator row in sbuf; chop straight into the
                        # [16,64] partition-parallel slot (sbuf->sbuf DMA)
                        dst = (dn16[h * 16:(h + 1) * 16, :] if h < 7
                               else dn16b[:, :])
                        nc.sync.dma_start(out=dst, in_=o_sb[h][D:DA, :])

                def emit_recip_chain(h0, nh):
                    # one batched DVE reciprocal over heads h0..h0+nh-1, then
                    # bounce through DRAM for the partition-broadcast reads
                    src, dst = ((dn16[h0 * 16:(h0 + nh) * 16, :],
                                 rcp16[h0 * 16:(h0 + nh) * 16, :])
                                if h0 < 7 else (dn16b[:, :], rcp16b[:, :]))
                    with nc.allow_low_precision(reason="denom ~1e3"):
                        nc.vector.reciprocal(dst, src)
                    nc.sync.dma_start(
                        out=rc_dram[h0:h0 + nh, :].rearrange(
                            "h (p c) -> (h p) c", c=64),
                        in_=dst)
                    for h in range(h0, h0 + nh):
                        row = rc_dram[h:h + 1, :]
                        bc = bass.AP(tensor=row.tensor, offset=row.offset,
                                     ap=[[0, D]] + [list(d) for d in row.ap[1:]])
                        nc.sync.dma_start(out=rbc[h], in_=bc)

                def emit_norm(h):
                    for off in (0, 512):
                        nc.vector.tensor_mul(o_norm[h][:, off:off + 512],
                                             o_sb[h][0:D, off:off + 512],
                                             rbc[h][:, off:off + 512])

                def proj_ps(g, w):
                    pool = mmps if g % 2 == 0 else stps
                    return pool.tile([128, w], F32, tag="mm" if g % 2 == 0
                                     else "st", name="ps_proj")

                def emit_proj_mms(ps, i, ci, hs):
                    isl = slice(i * 128, (i + 1) * 128)
                    off, w = ((0, 512), (512, E - 512))[ci]
                    for h in hs:
                        nc.tensor.matmul(ps, o_norm[h][:, isl],
                                         wp_sb[h][:, off:off + w],
                                         start=(h == 0), stop=(h == H - 1))

                def emit_proj_fin(ps, i, ci):
                    isl = slice(i * 128, (i + 1) * 128)
                    off, w = ((0, 512), (512, E - 512))[ci]
                    ot = outp.tile([128, w], F16, tag="out", name="out_sb")
                    nc.vector.tensor_copy(ot, ps)
                    nc.sync.dma_start(out=out[isl, off:off + w], in_=ot)

                # ---------------- main schedule ----------------
                for idx in range(4):
                    emit_qk_group(0, idx)

                ex_prev = None
                for h in range(H):
                    ex = []
                    for j in range(NT):
                        # side task first: keeps ready PE work ahead of the
                        # S^T matmuls, which wait on the previous casts
                        if h == 0:
                            emit_v_group(j)
                            if j in (3, 4, 5, 6):
                                emit_qk_group(1, j - 3)
                        else:
                            if j == 0:
                                emit_av_group(h - 1, ex_prev, 0)
                            elif j == 2:
                                emit_av_group(h - 1, ex_prev, 512)
                            elif h < H - 1 and j in (1, 3, 4, 5):
                                emit_qk_group(h + 1, {1: 0, 3: 1, 4: 2, 5: 3}[j])
                        js = slice(j * 128, (j + 1) * 128)
                        st = stps.tile([128, N], F32, tag="st", name="st_ps")
                        nc.tensor.matmul(st[:, 0:512], kT[h][0:D, js],
                                         qT[h][0:D, 0:512],
                                         start=True, stop=True)
                        nc.tensor.matmul(st[:, 512:1024], kT[h][0:D, js],
                                         qT[h][0:D, 512:1024],
                                         start=True, stop=True)
                        exj = exps.tile([128, N], F16, tag=f"ex{j}",
                                        name=f"ex{h}_{j}")
                        nc.scalar.activation(exj, st, EXP, scale=SCALE)
                        ex.append(exj)
                        if h == 7 and j == 3:
                            # heads 0-6 denominators are all staged by now
                            emit_recip_chain(0, 7)
                            for hh in range(7):
                                emit_norm(hh)
                    ex_prev = ex

                # drain: last head's AV; its normalization chain (reciprocal,
                # then a K=1 PE matmul broadcasts the reciprocal row across
                # the 96 d-partitions -- no DRAM bounce) is overlapped with
                # the first proj groups' head-0..6 accumulation passes
                emit_av_group(7, ex_prev, 0)
                emit_av_group(7, ex_prev, 512)
                with nc.allow_low_precision(reason="denom ~1e3"):
                    nc.vector.reciprocal(rcp16b, dn16b)
                nc.sync.dma_start(out=r7row, in_=rcp16b)  # [16,64] -> [1,1024]
                pre = []
                for g in range(4):
                    i, ci = g // 2, g % 2
                    ps = proj_ps(g, ((0, 512), (512, E - 512))[ci][1])
                    emit_proj_mms(ps, i, ci, range(7))
                    pre.append((ps, i, ci))
                for off in (0, 512):
                    bp = avps.tile([D, 512], F32, tag="av", name="bc_ps")
                    nc.tensor.matmul(bp, ones1[0:1, :],
                                     r7row[0:1, off:off + 512],
                                     start=True, stop=True)
                    nc.vector.tensor_mul(o_norm[7][:, off:off + 512],
                                         o_sb[7][0:D, off:off + 512], bp)
                for ps, i, ci in pre:
                    emit_proj_mms(ps, i, ci, [7])
                    emit_proj_fin(ps, i, ci)
                for g in range(4, 2 * NT):
                    i, ci = g // 2, g % 2
                    ps = proj_ps(g, ((0, 512), (512, E - 512))[ci][1])
                    emit_proj_mms(ps, i, ci, range(H))
                    emit_proj_fin(ps, i, ci)

    nc.compile()
    return nc


def _chop8(a):
    """[E, M] -> fp8 [128, ET, M] with row c*128+p at (p, c)."""
    import ml_dtypes
    return np.ascontiguousarray(
        a.reshape(ET, 128, -1).transpose(1, 0, 2)).astype(ml_dtypes.float8_e4m3)


def prep_weights(Wqkv, bqkv, Wproj, bproj):
    Wr = np.asarray(Wqkv, np.float32).reshape(E, H, D, 3)
    br = np.asarray(bqkv, np.float32).reshape(H, D, 3)
    wq_full = np.zeros((E, H * DP), np.float32)
    wk_full = np.zeros((E, H * DP), np.float32)
    wv_full = np.zeros((E, H * DA), np.float32)
    bq_full = np.zeros((DP, H), np.float32)
    for h in range(H):
        wq_full[:, h * DP:h * DP + D] = Wr[:, h, :, 0]
        wk_full[:, h * DP:h * DP + D] = Wr[:, h, :, 1]
        wv_full[:, h * DA:h * DA + D] = Wr[:, h, :, 2]
        bq_full[0:D, h] = br[h, :, 0]
    # host-side output bias: attn rows sum to 1, so attn@(v+bv) = attn@v + bv
    # and (o + bv_cat) @ Wproj + bproj = o @ Wproj + bp_eff
    bv_cat = br[:, :, 2].reshape(E)
    bp_eff = bv_cat @ np.asarray(Wproj, np.float64) + np.asarray(bproj, np.float64)
    return {
        "wq8": _chop8(wq_full),
        "wk8": _chop8(wk_full),
        "wv": wv_full.astype(np.float16),
        "wp": np.asarray(Wproj, np.float32).astype(np.float16),
        "bq": bq_full,
    }, bp_eff.astype(np.float32)


def make_in_maps(x, Wqkv, bqkv, Wproj, bproj):
    x = np.asarray(x, np.float32)
    shared, bp_eff = prep_weights(Wqkv, bqkv, Wproj, bproj)
    make_in_maps.bp_eff = bp_eff
    in_maps = []
    for b in range(B):
        xTb = np.ascontiguousarray(x[b].T)
        m = {"xT": xTb.astype(np.float16), "x8": _chop8(xTb)}
        m.update(shared)
        in_maps.append(m)
    return in_maps


_prog_cache = []


def kernel(x, Wqkv, bqkv, Wproj, bproj, _run_kwargs=None):
    from concourse.bass_utils import run_bass_kernel_spmd

    in_maps = make_in_maps(x, Wqkv, bqkv, Wproj, bproj)
    if not _prog_cache:
        _prog_cache.append(build_program())
    nc = _prog_cache[0]
    res = run_bass_kernel_spmd(nc, in_maps, core_ids=list(range(B)),
                               **(_run_kwargs or {}))
    out = np.stack([r["out"] for r in res.results], axis=0)
    out = out + make_in_maps.bp_eff
    if _run_kwargs:
        kernel.last_result = res
    return out
